# revision 20
# baseline (speedup 1.0000x reference)
"""AFNO spectral attention kernel for 8 TRN2 NeuronCores.

Math: the reference's rfft2 -> truncate -> per-block mode mix -> irfft2
collapses to a per-block real 224x224 matrix A_b applied along the W axis
(the H-direction FFT commutes with the mode mixing and cancels), plus a
bias-driven constant on the n_h==0 spatial rows. The final projection
folds into P = rescale*proj_w.T + I so the device only runs two matmul
stages:
  stage 1: Xs[r, w, c] = sum_w' X[r, w', c] * A_{b(c)}[w', w]
  stage 2: out[t, o]   = sum_c Xs[t, c] * P[c, o]  (+ rescale*proj_b)

Sharding: 100352 tokens = 8 cores x 12544 (56 complete image rows per
core, batch boundary lands exactly on the core-4 boundary). No
collectives needed.

Layout notes: x is host-padded along W' to 256 so each row-group loads
with a single rectangular DMA (keeps per-matmul semaphore wait sets
within the HW limit) and both contraction chunks are K=128. A and P ship
in one packed [128, 8192] constant so all weights arrive on one DMA
lane. Stage-1 PSUM blocks ([c96, w224]) are copied into stage-2's lhsT
layout with partition-shifted 32-wide DVE copies that assemble four
96-channel blocks into three 128-partition contraction chunks.
"""

import numpy as np
import ml_dtypes

import concourse.bass as bass
import concourse.mybir as mybir
import concourse.tile as tile
from concourse.bass_utils import run_bass_kernel_spmd

B, Hh, Ww, C = 2, 224, 224, 768
NB, BS, M = 8, 96, 96
NMODES = Ww // 2 + 1  # 113
N_CORES = 8
TOK = B * Hh * Ww  # 100352 total tokens
TOK_CORE = TOK // N_CORES  # 12544
ROWS_CORE = TOK_CORE // Ww  # 56 image rows per core
WP = 256  # padded W'
RG = 4  # image rows per group
GROUPS = ROWS_CORE // RG  # 14
TG = RG * Ww  # tokens per group = 896
TCH = TG // 128  # t-chunks of 128 per group = 7
A_COLS = 2 * NB * Ww  # 3584
P_COLS = 6 * C  # 4608
WC_COLS = A_COLS + P_COLS  # 8192

BF16 = ml_dtypes.bfloat16

_CACHE = {}


def _build_amat(block_weights, gates):
    """Per-block real [224, 224] spatial-W operator."""
    g = 1.0 / (1.0 + np.exp(-gates.astype(np.float64)))
    F = np.fft.rfft(np.eye(Ww), axis=1, norm="ortho")  # (224, 113)
    A = np.zeros((NB, Ww, Ww), np.float64)
    for b in range(NB):
        T = np.zeros((NMODES, NMODES), np.complex128)
        T[:M, :M] = g[b] * block_weights[b].astype(np.float64)
        for k in range(M, NMODES):
            T[k, k] = 1.0
        A[b] = np.fft.irfft(F @ T, n=Ww, axis=1, norm="ortho")
    return A, g


def _bias_const_rows(block_bias, g):
    """Constant added to spatial rows n_h == 0, per block: (NB, 224)."""
    rows = np.zeros((NB, Ww), np.float64)
    for b in range(NB):
        z = np.zeros(NMODES, np.complex128)
        z[:M] = g[b] * block_bias[b].astype(np.float64) * (1.0 + 1.0j)
        rows[b] = np.sqrt(Hh) * np.fft.irfft(z, n=Ww, norm="ortho")
    return rows


def _pack_weights(A, P):
    """[128, 8192] bf16: A chunks then assembled P chunks."""
    wc = np.zeros((128, WC_COLS), np.float32)
    for k in range(2):
        for b in range(NB):
            blk = A[b, k * 128 : min((k + 1) * 128, Ww), :]  # (128|96, 224)
            wc[: blk.shape[0], k * NB * Ww + b * Ww : k * NB * Ww + (b + 1) * Ww] = blk
    for k in range(6):
        q, kk = divmod(k, 3)
        wc[:, A_COLS + k * C : A_COLS + (k + 1) * C] = (
            P[q * 384 + kk * 128 : q * 384 + (kk + 1) * 128, :]
        )
    return wc.astype(BF16)


def _elide_redundant_waits(nc):
    """Drop per-instruction semaphore waits already implied by the
    instruction's other waits (transitively, via the wait chains of the
    instructions that perform the increments). Tile's sem assignment is
    per-proc minimal but not transitively minimal across procs, and
    walrus's per-instruction sync-command budget is tiny (matmul fits
    only one wait + one update)."""
    fn = nc.m.functions[0]
    implied = {}  # sem name -> [state dict after k-th increment]
    engine_state = {}  # engine -> folded state of prior instructions' waits

    def state_of(sem, v):
        lst = implied.get(sem)
        if not lst or v <= 0:
            return {}
        return lst[min(v, len(lst)) - 1]

    def fold(dst, src):
        for s, v in src.items():
            if dst.get(s, 0) < v:
                dst[s] = v

    for blk in fn.blocks:
        for inst in blk.instructions:
            si = inst.sync_info
            eng = getattr(inst, "engine", None)
            waits = list(si.on_wait or []) if si else []
            my = dict(engine_state.get(eng, {}))
            for w in waits:
                if w.wait_value is None:
                    continue
                fold(my, {w.ant_name: w.wait_value})
                fold(my, state_of(w.ant_name, w.wait_value))
            if len(waits) > 1 and all(w.wait_value is not None for w in waits):
                keep = []
                for w in waits:
                    others = dict(engine_state.get(eng, {}))
                    for w2 in waits:
                        if w2 is w:
                            continue
                        fold(others, {w2.ant_name: w2.wait_value})
                        fold(others, state_of(w2.ant_name, w2.wait_value))
                    if others.get(w.ant_name, -1) >= w.wait_value:
                        continue
                    keep.append(w)
                if len(keep) != len(waits):
                    si.on_wait = keep
            if eng is not None:
                engine_state[eng] = my
            for u in (si.on_update or []) if si else []:
                nm = u.ant_name
                lst = implied.setdefault(nm, [])
                prev = dict(lst[-1]) if lst else {}
                fold(prev, my)
                n = u.update_value or 1
                prev[nm] = len(lst) + n
                for _ in range(int(n)):
                    lst.append(prev)


def _build_nc():
    nc = bass.Bass("TRN2", target_bir_lowering=False)
    x_ext = nc.declare_dram_parameter(
        "x", [ROWS_CORE, WP, C], mybir.dt.bfloat16, isOutput=False
    )
    w_ext = nc.declare_dram_parameter(
        "wconst", [128, WC_COLS], mybir.dt.bfloat16, isOutput=False
    )
    out_ext = nc.declare_dram_parameter(
        "out", [TOK_CORE, C], mybir.dt.float32, isOutput=True
    )

    def a_sl(k, b):
        return slice(k * NB * Ww + b * Ww, k * NB * Ww + (b + 1) * Ww)

    def p_sl(k, lo, hi):
        return slice(A_COLS + k * C + lo, A_COLS + k * C + hi)

    with tile.TileContext(nc) as tc:
        with (
            tc.tile_pool(name="const", bufs=1) as const_pool,
            tc.tile_pool(name="xin", bufs=2) as x_pool,
            tc.tile_pool(name="st", bufs=2) as st_pool,
            tc.tile_pool(name="xs", bufs=2) as xs_pool,
            tc.tile_pool(name="gps", bufs=2, space="PSUM") as g_psum,
            tc.tile_pool(name="ops", bufs=2, space="PSUM") as o_psum,
            tc.tile_pool(name="osb", bufs=3) as out_pool,
        ):
            wc = const_pool.tile([128, WC_COLS], mybir.dt.bfloat16)
            nc.sync.dma_start(wc[:, :], w_ext[:, :])

            def load_x(g):
                xt = x_pool.tile([128, RG, 2, C], mybir.dt.bfloat16, tag="xin")
                src = x_ext[g * RG : (g + 1) * RG, :, :]
                nc.gpsimd.dma_start(
                    xt[:, :, :, :], src.rearrange("r (k p) c -> p r k c", p=128)
                )
                return xt

            # natural assembly pieces (4 blocks -> 3 x 128-chunks):
            # (m, src_lo, src_hi, dst_chunk, dst_part)
            ASM = [
                (0, 0, 96, 0, 0),
                (1, 0, 32, 0, 96), (1, 32, 64, 1, 0), (1, 64, 96, 1, 32),
                (2, 0, 32, 1, 64), (2, 32, 64, 1, 96), (2, 64, 96, 2, 0),
                (3, 0, 32, 2, 32), (3, 32, 64, 2, 64), (3, 64, 96, 2, 96),
            ]

            def stage1(g, xt):
                """returns xs tile [128, 6, TG] bf16 (assembled chunks)"""
                # flat tiles with one spare gate element at the end so the
                # gates never overlap real data (no same-engine WAW waits)
                st = st_pool.tile([96, NB * TG + 1], mybir.dt.bfloat16, tag="st")
                # DVE gate: absorbs st's WAR on the gpsimd assembly readers
                nc.vector.tensor_copy(st[0:1, NB * TG :], wc[0:1, 0:1])
                xs = xs_pool.tile([128, 6 * TG + 1], mybir.dt.bfloat16, tag="xs")
                # gpsimd gate: absorbs xs's WAR on stage-2 PE readers
                nc.gpsimd.tensor_copy(xs[0:1, 6 * TG :], wc[0:1, 0:1])
                for b in range(NB):
                    # [96, 2 banks, 512]: rows r at (r//2, (r%2)*224)
                    pg = g_psum.tile([96, 2, 512], mybir.dt.float32, tag="gps")
                    for r in range(RG):
                        o = (r % 2) * Ww
                        for k in range(2):
                            nc.tensor.matmul(
                                pg[:, r // 2, o : o + Ww],
                                lhsT=xt[0:128, r, k, b * BS : (b + 1) * BS],
                                rhs=wc[0:128, a_sl(k, b)],
                                start=(k == 0),
                                stop=(k == 1),
                            )
                    nc.vector.tensor_copy(
                        st[:, b * TG : (b + 1) * TG], pg[:, :, 0 : 2 * Ww]
                    )
                for q in range(2):
                    for (m, lo, hi, ck, dp) in ASM:
                        nc.gpsimd.tensor_copy(
                            xs[dp : dp + hi - lo,
                               (3 * q + ck) * TG : (3 * q + ck + 1) * TG],
                            st[lo:hi, (4 * q + m) * TG : (4 * q + m + 1) * TG],
                        )
                return xs

            def stage2(g, xs):
                ots = []
                for j in range(TCH):
                    po1 = o_psum.tile([128, 512], mybir.dt.float32, tag="po1")
                    po2 = o_psum.tile([128, 256], mybir.dt.float32, tag="po2")
                    for k in range(6):
                        lhsT = xs[0:128, k * TG + j * 128 : k * TG + (j + 1) * 128]
                        nc.tensor.matmul(
                            po1[:, :],
                            lhsT=lhsT,
                            rhs=wc[0:128, p_sl(k, 0, 512)],
                            start=(k == 0),
                            stop=(k == 5),
                        )
                        nc.tensor.matmul(
                            po2[:, :],
                            lhsT=lhsT,
                            rhs=wc[0:128, p_sl(k, 512, 768)],
                            start=(k == 0),
                            stop=(k == 5),
                        )
                    # one spare column: the 1-element gate copy absorbs
                    # ot's slot-free (DMA) wait on the ACT queue so the
                    # real copies only carry the PE wait (walrus allows
                    # one wait per Activation), without WAW overlap.
                    ot = out_pool.tile([128, C + 1], mybir.dt.float32, tag="osb")
                    nc.scalar.copy(ot[0:1, C : C + 1], wc[0:1, 0:1])
                    nc.scalar.copy(ot[:, 0:512], po1[:, :])
                    nc.scalar.copy(ot[:, 512:768], po2[:, :])
                    t0 = g * TG + j * 128
                    nc.sync.dma_start(out_ext[t0 : t0 + 128, :], ot[:, 0:C])
                    ots.append(ot)
                return ots

            # software pipeline: stage1(g) then stage2(g-1) in PE program order
            xs_prev = None
            xt = load_x(0)
            for g in range(GROUPS):
                xs_cur = stage1(g, xt)
                if g + 1 < GROUPS:
                    xt = load_x(g + 1)
                if xs_prev is not None:
                    stage2(g - 1, xs_prev)
                xs_prev = xs_cur
            last_ots = stage2(GROUPS - 1, xs_prev)
            # tail joins: tiny ACT writes into the last out tiles make the
            # ACT queue observe the final out-DMA completions (WAR), so the
            # kernel-tail Drain's DMA-lane waits become implied and are
            # elided (walrus allows only one wait on Drain).
            for ot in last_ots[-3:]:
                nc.scalar.copy(ot[0:1, 0:1], wc[0:1, 0:1])

    _elide_redundant_waits(nc)
    return nc


def kernel(x, block_weights, block_bias, gates, proj_w, proj_b, rescale):
    x = np.asarray(x)
    A, g = _build_amat(np.asarray(block_weights), np.asarray(gates))
    P = float(rescale) * np.asarray(proj_w, np.float64).T + np.eye(C)
    w_dev = _pack_weights(A, P)

    # pad W' 224 -> 256 with zeros; shard 56 rows per core
    x_rows = x.reshape(TOK // Ww, Ww, C)
    x_pad = np.zeros((TOK // Ww, WP, C), BF16)
    x_pad[:, :Ww, :] = x_rows.astype(BF16)

    if "nc" not in _CACHE:
        _CACHE["nc"] = _build_nc()
    nc = _CACHE["nc"]

    in_maps = []
    for i in range(N_CORES):
        in_maps.append(
            {
                "x": x_pad[i * ROWS_CORE : (i + 1) * ROWS_CORE],
                "wconst": w_dev,
            }
        )
    res = run_bass_kernel_spmd(
        nc,
        in_maps,
        core_ids=list(range(N_CORES)),
        trace=bool(_CACHE.get("trace", False)),
        **_CACHE.get("trace_kwargs", {}),
    )
    _CACHE["last_results"] = res
    out = np.concatenate([r["out"] for r in res.results], axis=0)
    out = out.reshape(B, Hh * Ww, C).astype(np.float32)

    # host-side constant corrections (zero for the reference inputs)
    bb = np.asarray(block_bias)
    pb = np.asarray(proj_b)
    if np.any(bb) or np.any(pb):
        const = np.zeros((Hh * Ww, C), np.float64)
        if np.any(bb):
            rows = _bias_const_rows(bb, g)  # (NB, 224)
            cr = np.zeros((Ww, C), np.float64)
            for b in range(NB):
                cr[:, b * BS : (b + 1) * BS] = rows[b][:, None]
            # affects tokens with n_h == 0: tokens 0..223 of each batch image
            const[0:Ww, :] = cr @ P  # x_const goes through out = x_const @ P
        add = const[None, :, :] + float(rescale) * pb.astype(np.float64)[None, None, :]
        out = (out.astype(np.float64) + add).astype(np.float32)
    return out
